# revision 11
# baseline (speedup 1.0000x reference)
"""fp8 quantized matmul y = fp8(x) @ fp8(W)^T on 8 Trainium2 NeuronCores.

Reference semantics: x[M,K] f32 and W[N,K] f32 are each cast to
float8_e4m3fn (OCP, round-to-nearest-even) and the matmul accumulates in
fp32.  The cast is a pure element-wise dtype conversion, done here on the
host with ml_dtypes (bit-identical to the reference's jax cast for the
value range involved: |x| < 16, |W| <= 2^-6, both far below 240 where the
OCP and IEEE e4m3 encodings coincide).

Sharding: 2D (4 x 2) — core c computes the [2048, 2048] block
y[(c//2)*2048 :, (c%2)*2048 :] from a 2048-row x shard (8 MiB fp8) and a
2048-col W shard (8 MiB fp8), both SBUF-resident.  This minimizes
per-core (and aggregate) HBM traffic — 16 MiB/core vs 20 for 8x1
data-parallel — which matters because all 8 cores pull their inputs
concurrently at ramp, and it gives each 2 MiB W column a ~5x delivery
margin over its consumption period (55 us) so the stream never waits on
a column edge.

Device kernel: fp8 DoubleRow matmuls.  The PE streams back-to-back at the
fp8 peak (512 cycles per [128,512]-out matmul, LDWEIGHTS pipelined under
the previous stream), so the kernel is pure compute-roofline: 1024 MMs x
~216 ns = 221 us of PE stream.  Everything else is ramp/tail engineering:

 * the first W column is trickled on the ACT HWDGE ring while the x
   slivers ride the SP ring — the two physical rings deliver the first
   128 KiB of each operand concurrently, so tile (0,0) starts ~8-10 us
   in instead of ~19 (single-ring FIFO would serialize x behind W).
 * first slivers are small (x: 4x128 KiB; W col 0: kq=[2,2,4,8,8,8]
   subtile groups) so the first matmul's dependencies land early; later
   columns stream as whole 2 MiB DMAs with a ~10x time margin.
 * a handful of tiny N=128 warmup matmuls on one memset tile bridge the
   startup barrier -> first-data window and release the HAM clock gate /
   PE p-state ramp before real matmuls begin.
 * n-major tile order (all 8 m-tiles per W column before the next column)
   keeps the mid-stream fresh-W appetite at ~72 GB/s — no starvation.
 * DVE evacuates PSUM -> SBUF as fp16 (halving store traffic; ~5e-4
   relative noise, far under the 2e-2 gate); stores ride the ACT ring
   behind only the 2 MiB of w0 slivers; the last two tiles are evicted
   in halves/quarters so the final DVE copy and store pipeline into the
   kernel-tail drain.

Host-side layouts are pre-transposed so every DMA is a large contiguous
per-partition transfer:
  xt[mt, p, kt, m] = fp8(x_shard)[mt*128 + m, kt*128 + p]   (4 MiB/core)
  wt[nt, p, kt, n] = fp8(W)[nt*512 + n, kt*128 + p]         (16 MiB)
The [p, kt, cols] SBUF tiles feed nc.tensor.matmul sliced
[:, 2t:2t+2, :] — the DoubleRow contraction pair is (kt*128+p) over two
consecutive kt subtiles, identically on both operands.
"""

import numpy as np
import ml_dtypes

P = 128          # partitions
N_CORES = 8
GM, GN = 4, 2              # core grid: 4-way over M, 2-way over N
M, K, N = 8192, 4096, 4096
MC = M // GM               # 2048 rows of x per core
NC = N // GN               # 2048 out-features per core
MT = MC // P               # 16 m-tiles per core
KT = K // P                # 32 k-subtiles
NB = 512                   # psum bank width (f32)
NT = NC // NB              # 4 n-tiles per core

_NC_CACHE = {}


N_WARMUP = 30    # tiny N=128 PE matmuls bridging startup barrier -> stream
W0_KQ = (8, 8, 8, 8)         # kt-subtiles per w column-0 sliver


def _emit(nc, tc, mybir, X, W, Y, mt_n, nt_n, kt_n, nb):
    fp8 = mybir.dt.float8e4
    f32 = mybir.dt.float32
    f16 = mybir.dt.float16
    import contextlib

    n_pairs = kt_n // 2
    assert sum(W0_KQ) == kt_n and all(k % 2 == 0 for k in W0_KQ)
    # pair-offset of each w0 sliver group
    w0_pair_start = []
    acc = 0
    for k in W0_KQ:
        w0_pair_start.append(acc)
        acc += k // 2

    with contextlib.ExitStack() as ctx:
        warm = ctx.enter_context(tc.tile_pool(name="warm", bufs=1))
        xpool = ctx.enter_context(tc.tile_pool(name="xpool", bufs=1))
        wpool = ctx.enter_context(tc.tile_pool(name="wpool", bufs=1))
        spool = ctx.enter_context(tc.tile_pool(name="spool", bufs=8))
        ppool = ctx.enter_context(
            tc.tile_pool(name="ppool", bufs=4, space="PSUM")
        )

        # PE warmup on one tiny memset tile (used as both operands):
        # occupies the tensor engine from the end of the startup barrier
        # until the first input DMAs land, so the HAM clock gate is
        # released and the p-state ramped before real matmuls begin.
        # The memset rides the vector engine (free earliest after the
        # prologue) so the first warmup starts ~6us in.
        wm = warm.tile([P, 2, P], fp8, name="wm", tag="wm")
        nc.vector.memset(wm, 0.0)
        wm_ps = ppool.tile([P, P], f32, name="wm_ps", tag="ps")
        for _ in range(N_WARMUP):
            nc.tensor.matmul(
                wm_ps,
                wm,
                wm,
                start=True,
                stop=True,
                perf_mode=mybir.MatmulPerfMode.DoubleRow,
            )

        # Input loads.  x slivers/tiles ride the SP HWDGE ring; the first
        # W column's slivers ride the ACT ring (otherwise idle until the
        # first PSUM eviction ~13us in) so the first 128 KiB of x and W
        # transfer CONCURRENTLY — the rings are FIFO per engine, so a
        # single-ring order would serialize them.  Later W columns are
        # whole-nt 2 MiB DMAs on the SP ring, landing with a ~10x margin.
        xt = [None] * mt_n          # whole x tiles
        wt = [None] * nt_n          # whole W columns (nt >= 1)
        w0g = [None] * len(W0_KQ)   # W column-0 slivers

        def load_x(mt):
            t = xpool.tile([P, kt_n, P], fp8, name=f"xt{mt}", tag=f"xt{mt}")
            nc.sync.dma_start(out=t, in_=X[mt, :, :, :])
            xt[mt] = t

        def load_w(nt):
            t = wpool.tile([P, kt_n, nb], fp8, name=f"wt{nt}", tag=f"wt{nt}")
            nc.sync.dma_start(out=t, in_=W[nt, :, :, :])
            wt[nt] = t

        def load_w0(g):
            kq = W0_KQ[g]
            k0 = 2 * w0_pair_start[g]
            t = wpool.tile([P, kq, nb], fp8, name=f"w0g{g}", tag=f"w0g{g}")
            nc.scalar.dma_start(out=t, in_=W[0, :, k0 : k0 + kq, :])
            w0g[g] = t

        for g in range(len(W0_KQ)):
            load_w0(g)
        for mt in range(mt_n):
            load_x(mt)
        for nt in range(1, nt_n):
            load_w(nt)

        def x_slice(mt, t2):
            return xt[mt][:, 2 * t2 : 2 * t2 + 2, :]

        def w_slice(nt, t2):
            if nt == 0:
                # find sliver group containing pair t2
                g = len(W0_KQ) - 1
                while w0_pair_start[g] > t2:
                    g -= 1
                l = t2 - w0_pair_start[g]
                return w0g[g][:, 2 * l : 2 * l + 2, :]
            return wt[nt][:, 2 * t2 : 2 * t2 + 2, :]

        def emit_tile(nt, mt, n_off, n_len):
            ps = ppool.tile([P, n_len], f32, name="ps", tag="ps")
            for t2 in range(n_pairs):
                nc.tensor.matmul(
                    ps,
                    x_slice(mt, t2),
                    w_slice(nt, t2)[:, :, n_off : n_off + n_len],
                    start=(t2 == 0),
                    stop=(t2 == n_pairs - 1),
                    perf_mode=mybir.MatmulPerfMode.DoubleRow,
                )
            st = spool.tile([P, n_len], f16, name="st", tag="st")
            nc.vector.tensor_copy(out=st, in_=ps)
            # outputs ride the ACT HWDGE ring, behind only the 2 MiB of
            # w0 slivers — they never queue behind the bulk input loads
            # on the SP ring
            nc.scalar.dma_start(
                out=Y[
                    mt * P : (mt + 1) * P,
                    nt * nb + n_off : nt * nb + n_off + n_len,
                ],
                in_=st,
            )

        for nt in range(nt_n):
            for mt in range(mt_n):
                last = nt == nt_n - 1 and mt == mt_n - 1
                second_last = nt == nt_n - 1 and mt == mt_n - 2
                if last:
                    # quarter the very last output tile so each PSUM
                    # eviction + store overlaps the next quarter's
                    # matmuls instead of sitting exposed before the
                    # kernel-tail drain
                    q = nb // 4
                    for i in range(4):
                        emit_tile(nt, mt, i * q, q)
                elif second_last:
                    emit_tile(nt, mt, 0, nb // 2)
                    emit_tile(nt, mt, nb // 2, nb - nb // 2)
                else:
                    emit_tile(nt, mt, 0, nb)


def _build(mt_n=MT, nt_n=NT, kt_n=KT, nb=NB, hw=True):
    import concourse.bacc as bacc
    import concourse.mybir as mybir
    import concourse.tile as tile
    from concourse.bass_interp import get_hw_module

    nc = bacc.Bacc("TRN2", target_bir_lowering=False, debug=False)
    X = nc.dram_tensor(
        "xt", [mt_n, P, kt_n, P], mybir.dt.float8e4, kind="ExternalInput"
    ).ap()
    W = nc.dram_tensor(
        "wt", [nt_n, P, kt_n, nb], mybir.dt.float8e4, kind="ExternalInput"
    ).ap()
    Y = nc.dram_tensor(
        "y", [mt_n * P, nt_n * nb], mybir.dt.float16, kind="ExternalOutput"
    ).ap()
    with tile.TileContext(nc) as tc:
        _emit(nc, tc, mybir, X, W, Y, mt_n, nt_n, kt_n, nb)
    nc.compile()
    if hw:
        nc.m = get_hw_module(nc.m)
    return nc


def _get_nc():
    if "nc" not in _NC_CACHE:
        _NC_CACHE["nc"] = _build()
    return _NC_CACHE["nc"]


def _quantize(a):
    # OCP e4m3fn RNE cast (matches jax astype), then reinterpret as the
    # IEEE e4m3 dtype the BIR tensor declares (identical bits below 240).
    return a.astype(ml_dtypes.float8_e4m3fn).view(ml_dtypes.float8_e4m3)


def _in_maps(x, W):
    xq = _quantize(np.ascontiguousarray(x))
    wq = _quantize(np.ascontiguousarray(W))
    # per N-shard: wt[nt, p, kt, n] = w_shard[nt*NB + n, kt*P + p]
    wts = []
    for j in range(GN):
        ws = wq[j * NC : (j + 1) * NC]
        wts.append(
            np.ascontiguousarray(ws.reshape(NT, NB, KT, P).transpose(0, 3, 2, 1))
        )
    xts = []
    for i in range(GM):
        xc = xq[i * MC : (i + 1) * MC]
        # xt[mt, p, kt, m] = xc[mt*P + m, kt*P + p]
        xts.append(
            np.ascontiguousarray(xc.reshape(MT, P, KT, P).transpose(0, 3, 2, 1))
        )
    return [{"xt": xts[c // GN], "wt": wts[c % GN]} for c in range(N_CORES)]


def _ensure_axon_ntff_hook():
    # Under axon, run_bass_kernel_spmd(trace=True) imports
    # antenv.axon_hooks, which some images lack even though the boot
    # machinery that implements the hook is present.  Register a shim so
    # tracing degrades gracefully instead of raising.
    import sys

    if "antenv.axon_hooks" in sys.modules:
        return
    try:
        from concourse._compat import axon_active

        if not axon_active():
            return
        import importlib.util

        if importlib.util.find_spec("antenv.axon_hooks") is not None:
            return
        import types

        import antenv

        hook = None
        try:
            import trn_agent_boot.trn_boot as _tb

            hook = _tb._ntff_profile_via_ctypes("/opt/axon/libaxon_pjrt.so")
        except Exception:
            hook = None
        mod = types.ModuleType("antenv.axon_hooks")
        mod._hook = hook
        mod.get_axon_ntff_profile_hook = lambda: mod._hook
        def _set(h):
            mod._hook = h
        mod.set_axon_ntff_profile_hook = _set
        antenv.axon_hooks = mod
        sys.modules["antenv.axon_hooks"] = mod
    except Exception:
        pass


def _run(in_maps, trace=False):
    from concourse.bass_utils import run_bass_kernel_spmd

    _ensure_axon_ntff_hook()
    nc = _get_nc()
    return run_bass_kernel_spmd(
        nc, in_maps, core_ids=list(range(len(in_maps))), trace=trace
    )


def kernel(x, W):
    res = _run(_in_maps(x, W))
    rows = [
        np.concatenate(
            [res.results[i * GN + j]["y"] for j in range(GN)], axis=1
        )
        for i in range(GM)
    ]
    return np.concatenate(rows, axis=0).astype(np.float32, copy=False)
